# revision 29
# baseline (speedup 1.0000x reference)
"""GCMC GraphConv kernel for 8 Trainium2 NeuronCores.

Computation:  out = ci * segment_sum((input_feat @ weight * cj)[src], dst)

Strategy v3 (aggregate-then-transform, dst-sharded, crossing-split tiles):
  - Algebra: out = ci * (A^T (cj*X)) W  — aggregate raw (cj-scaled) X rows per
    destination and multiply by W once per dst block at the end.
  - Nodes are 1D-partitioned by destination: core c owns 98 dst blocks of
    128 rows, assigned by sorted edge count for balance.
  - The per-edge gather (SWDGE dma_gather, 256B rows, 4 queues, 1024-desc
    calls) is descriptor-execution bound at ~2.6 ns/desc aggregate — so the
    dominant cost is proportional to the PADDED edge count.  v2 padded every
    (window, slot) group to a multiple of 128 (+24%).  v3 pads each group
    only to the max count over the 8 cores (+~6%) and lets matmul tiles span
    two adjacent slots: a "crossing" tile issues TWO scatter matmuls, one per
    slot, with complementary one-hot masks (rows of the other slot get
    dstb=-1 and match nothing).  The crossing structure is canonical (derived
    from the shared K[w][s] = max_c counts), so the SPMD program is identical
    across cores; only the idx/dstb data differs.
  - Scatter is a one-hot matmul accumulated TRANSPOSED into PSUM per slot,
    flushed into accT [128 feat, 98*128 dst] f32 after each (window, slot).
  - v4: the one-hots are NOT built on-device (the DVE is_equal builds ran at
    1x rate due to the broadcast operand and were a 73%-busy co-bottleneck
    that also backpressured the gather pipeline).  Instead the host
    precomputes them as fp8_e4m3 streams (128 B/row, 27 MB/core) DMAed from
    HBM; the scatter matmul runs mixed bf16 x fp8 (legal on TRN2's PE).
  - Epilogue per dst block: matmul(lhsT=accT block, rhs=W) -> [128, 64],
    scale by ci, store.
"""

import dataclasses
import math

import numpy as np
import ml_dtypes

import concourse.bacc as bacc
import concourse.mybir as mybir
import concourse.tile as tile
from concourse.bass_utils import run_bass_kernel_spmd

BF16 = ml_dtypes.bfloat16
F8 = ml_dtypes.float8_e4m3
P = 128
NCORES = 8
D_IN = 128


@dataclasses.dataclass(frozen=True)
class Cfg:
    N: int = 100000
    D_OUT: int = 64
    NWIN: int = 4            # src windows; N/NWIN must be < 32768 (int16 idx)
    MAX_CHUNK_TILES: int = 8   # gather chunk (1024 descs = SWDGE ring capacity)
    OH_GROUP: int = 64       # tiles per one-hot stream load (8 KB/partition)
    NQUEUES: int = 4         # SWDGE queues; round-robin -> concurrent rings
    SCRATCH: int = 32768     # dynamic DMA descriptor carveout (bytes/partition)

    @property
    def n_loc(self):
        return self.N // NCORES

    @property
    def nblk(self):
        return math.ceil(self.n_loc / P)

    @property
    def win(self):
        return self.N // self.NWIN


CFG = Cfg()


@dataclasses.dataclass
class Plan:
    """Canonical (core-independent) program structure."""
    off: np.ndarray        # [NWIN, nblk+1] canonical subgroup offsets
    wtot: np.ndarray       # [NWIN] padded window sizes (x128)
    ntiles: np.ndarray     # [NWIN] tiles per window
    tile_sa: list          # per window: [T_w] primary slot of each tile
    tile_cross: list       # per window: [T_w] bool, tile spans sa and sa+1
    touches: list          # per window: {slot: [(tile, 'A'|'B')]}


def make_plan(cfg: Cfg, K: np.ndarray) -> Plan:
    nblk = cfg.nblk
    off = np.zeros((cfg.NWIN, nblk + 1), dtype=np.int64)
    off[:, 1:] = np.cumsum(K, axis=1)
    wtot = ((off[:, -1] + P - 1) // P) * P
    ntiles = wtot // P
    tile_sa, tile_cross, touches = [], [], []
    for w in range(cfg.NWIN):
        assert (K[w] >= P).all(), "subgroup smaller than a tile"
        t = int(ntiles[w])
        pos0 = np.arange(t) * P
        pos1 = np.minimum(pos0 + P - 1, off[w, -1] - 1)
        sa = np.clip(np.searchsorted(off[w], pos0, side="right") - 1,
                     0, nblk - 1)
        sb = np.clip(np.searchsorted(off[w], pos1, side="right") - 1,
                     0, nblk - 1)
        assert (sb - sa <= 1).all() and (sb >= sa).all()
        cross = sb > sa
        tile_sa.append(sa)
        tile_cross.append(cross)
        tch = {s: [] for s in range(nblk)}
        for ti in range(t):
            tch[int(sa[ti])].append((ti, "A"))
            if cross[ti]:
                tch[int(sa[ti]) + 1].append((ti, "B"))
        touches.append(tch)
    return Plan(off, wtot, ntiles, tile_sa, tile_cross, touches)


# ---------------------------------------------------------------- host prep

def shard_edges(cfg: Cfg, src, dst):
    """Route and sort edges; build canonical plan + per-core padded arrays.

    Returns (plan, per_core, block_of).
    """
    nblk, win, nw = cfg.nblk, cfg.win, cfg.NWIN
    src = np.asarray(src, dtype=np.int64)
    dst = np.asarray(dst, dtype=np.int64)
    gb = dst >> 7
    dstb = (dst & 127).astype(np.float32)
    wine = src // win
    src_loc = (src - wine * win).astype(np.int16)

    nblk_g = NCORES * nblk
    bc = np.bincount(gb, minlength=nblk_g)
    order = np.argsort(-bc, kind="stable")
    block_of = np.empty((NCORES, nblk), dtype=np.int64)
    block_core = np.empty(nblk_g, dtype=np.int64)
    block_slot = np.empty(nblk_g, dtype=np.int64)
    # Within each octet, give the largest block to the core with the
    # smallest running edge total (HW time is the max over cores).
    core_tot = np.zeros(NCORES, dtype=np.int64)
    for s in range(nblk):
        grp = order[s * NCORES:(s + 1) * NCORES]   # descending size
        by_load = np.argsort(core_tot, kind="stable")
        block_of[by_load, s] = grp
        core_tot[by_load] += bc[grp]
        block_core[grp] = by_load
        block_slot[grp] = s

    core = block_core[gb]
    slot = block_slot[gb]

    gid = (core * nw + wine) * nblk + slot
    counts = np.bincount(gid, minlength=NCORES * nw * nblk)
    counts = counts.reshape(NCORES, nw, nblk)
    K = counts.max(axis=0)                    # [NWIN, nblk] canonical sizes
    plan = make_plan(cfg, K)

    per_core = []
    for c in range(NCORES):
        m = core == c
        sl, db, we, bl = src_loc[m], dstb[m], wine[m], slot[m]
        key = we * nblk + bl
        o = np.argsort(key * (win + 1) + sl, kind="stable")
        ks = key[o]
        gcnt = np.bincount(ks, minlength=nw * nblk)
        gstart = np.concatenate([[0], np.cumsum(gcnt)[:-1]])
        within = np.arange(ks.size) - gstart[ks]
        wsel, ssel = ks // nblk, ks % nblk
        pos = plan.off[wsel, ssel] + within
        sls, dbs = sl[o], db[o].astype(np.int64)
        maps = {}
        for w in range(nw):
            n = int(plan.wtot[w])
            nt = n // P
            ia = np.zeros(n, dtype=np.int16)          # pad -> row 0
            oa = np.zeros((n, P), dtype=np.uint8)     # fp8 one-hot bits
            sel = wsel == w
            p, s_e, d_e = pos[sel], ssel[sel], dbs[sel]
            ia[p] = sls[sel]
            sa_of = plan.tile_sa[w][p // P]
            in_a = s_e == sa_of
            oa[p[in_a], d_e[in_a]] = 0x38             # fp8_e4m3 1.0
            # B-stream: rows whose slot is the tile's secondary slot
            cross = plan.tile_cross[w]
            cidx = np.cumsum(cross) - 1
            ncross = int(cross.sum())
            ob = np.zeros((max(ncross, 1) * P, P), dtype=np.uint8)
            nb = ~in_a
            tt = p[nb] // P
            assert (s_e[nb] == sa_of[nb] + 1).all()
            assert cross[tt].all()
            ob[cidx[tt] * P + (p[nb] % P), d_e[nb]] = 0x38
            maps[f"idx{w}"] = np.ascontiguousarray(
                np.tile(ia.reshape(-1, 16).T, (8, 1)))
            # [rows, cols] -> [128 row-in-tile, tiles*128 cols]
            maps[f"ohA{w}"] = np.ascontiguousarray(
                oa.reshape(nt, P, P).transpose(1, 0, 2)
                .reshape(P, nt * P)).view(F8)
            maps[f"ohB{w}"] = np.ascontiguousarray(
                ob.reshape(-1, P, P).transpose(1, 0, 2)
                .reshape(P, -1)).view(F8)
        per_core.append(maps)
    return plan, per_core, block_of


def host_inputs(cfg: Cfg, input_feat, weight, cj, ci, block_of):
    """Shared (replicated) device inputs + per-core civ (slot layout)."""
    N, nblk = cfg.N, cfg.nblk
    xs = (np.asarray(input_feat, dtype=np.float32)
          * np.asarray(cj, dtype=np.float32)).astype(BF16)
    xw = np.ascontiguousarray(xs)
    wgt = np.ascontiguousarray(np.asarray(weight, dtype=np.float32))
    cip = np.zeros(NCORES * nblk * P, dtype=np.float32)
    cif = np.asarray(ci, dtype=np.float32).reshape(-1)
    cip[:N] = cif
    cip = cip.reshape(NCORES * nblk, P)
    civs = [np.ascontiguousarray(cip[block_of[c]].T) for c in range(NCORES)]
    return {"xw": xw, "wgt": wgt}, civs


# ---------------------------------------------------------------- device IR

def build_nc(cfg: Cfg, plan: Plan):
    f32, bf16, i16 = mybir.dt.float32, mybir.dt.bfloat16, mybir.dt.int16
    f8 = mybir.dt.float8e4
    dout, nblk, win = cfg.D_OUT, cfg.nblk, cfg.win
    ntiles = [int(plan.ntiles[w]) for w in range(cfg.NWIN)]
    ncross = [int(plan.tile_cross[w].sum()) for w in range(cfg.NWIN)]

    nc = bacc.Bacc("TRN2", target_bir_lowering=False, debug=False,
                   num_swdge_queues=cfg.NQUEUES,
                   dynamic_dma_scratch_size=cfg.SCRATCH)
    xw = nc.dram_tensor("xw", [cfg.N, D_IN], bf16, kind="ExternalInput")
    wgt = nc.dram_tensor("wgt", [D_IN, dout], f32, kind="ExternalInput")
    civ = nc.dram_tensor("civ", [P, nblk], f32, kind="ExternalInput")
    idx_t = [nc.dram_tensor(f"idx{w}", [P, ntiles[w] * 8], i16,
                            kind="ExternalInput") for w in range(cfg.NWIN)]
    oha_t = [nc.dram_tensor(f"ohA{w}", [P, ntiles[w] * P], f8,
                            kind="ExternalInput") for w in range(cfg.NWIN)]
    ohb_t = [nc.dram_tensor(f"ohB{w}", [P, max(ncross[w], 1) * P], f8,
                            kind="ExternalInput") for w in range(cfg.NWIN)]
    # partition-major output: [dst-in-block, slot*dout] — contiguous per
    # partition so the final store is 1 descriptor/partition; the host
    # transposes back (it unpermutes blocks anyway).
    out_t = nc.dram_tensor("out", [P, nblk * dout], f32,
                           kind="ExternalOutput")

    # per-window slot bookkeeping: first/last touch per slot
    first_touch = []   # per window: {(tile, which): slot}  -> start flag
    last_touch = []
    for w in range(cfg.NWIN):
        ft, lt = {}, {}
        for s, tl in plan.touches[w].items():
            ft[(tl[0][0], tl[0][1])] = s
            lt[(tl[-1][0], tl[-1][1])] = s
        first_touch.append(ft)
        last_touch.append(lt)

    with tile.TileContext(nc) as tc:
        with (
            tc.tile_pool(name="const", bufs=1) as cpool,
            tc.tile_pool(name="idx", bufs=4) as ipool,
            tc.tile_pool(name="msg", bufs=12) as mpool,
            tc.tile_pool(name="oh", bufs=3) as opool,
            tc.tile_pool(name="ohb", bufs=3) as obpool,
            tc.tile_pool(name="ps", bufs=4, space="PSUM") as pspool,
            tc.tile_pool(name="pso", bufs=4, space="PSUM") as psopool,
            tc.tile_pool(name="acc", bufs=1) as apool,
        ):
            wgt_sb = cpool.tile([P, dout], f32, tag="wgt")
            nc.sync.dma_start(out=wgt_sb[:], in_=wgt[:])
            ci_sb = cpool.tile([P, nblk], f32, tag="ci")
            nc.sync.dma_start(out=ci_sb[:], in_=civ[:])
            accT = apool.tile([P, nblk * P], f32, tag="accT")
            nc.vector.memset(accT[:], 0.0)
            out_sb = apool.tile([P, nblk * dout], f32, tag="out")

            ps_open = {}

            def emit_epilogue(s):
                pso = psopool.tile([P, dout], f32, tag="pso", name="pso")
                nc.tensor.matmul(
                    out=pso[:],
                    lhsT=accT[:, s * P:(s + 1) * P],
                    rhs=wgt_sb[:],
                    start=True, stop=True)
                nc.scalar.mul(
                    out_sb[:, s * dout:(s + 1) * dout],
                    pso[:],
                    ci_sb[:, s:s + 1])
                # store completed output ranges eagerly (shrinks the tail)
                if s in (31, 63, 89, nblk - 1):
                    q0 = {31: 0, 63: 32, 89: 64, nblk - 1: 90}[s]
                    nc.sync.dma_start(
                        out=out_t[:, q0 * dout:(s + 1) * dout],
                        in_=out_sb[:, q0 * dout:(s + 1) * dout])

            def emit_mm(w, t, which, s, msg, t0, oh, ohcol):
                start = first_touch[w].get((t, which)) == s
                stop = last_touch[w].get((t, which)) == s
                if start:
                    ps_open[s] = pspool.tile([P, P], f32, tag="psT",
                                             name="psT")
                ps = ps_open[s]
                nc.tensor.matmul(
                    out=ps[:],
                    lhsT=msg[:, (t - t0) * D_IN:(t - t0 + 1) * D_IN],
                    rhs=oh[:, ohcol * P:(ohcol + 1) * P],
                    start=start, stop=stop)
                if stop:
                    nc.vector.tensor_add(
                        out=accT[:, s * P:(s + 1) * P],
                        in0=accT[:, s * P:(s + 1) * P],
                        in1=ps[:])
                    del ps_open[s]
                    if w == cfg.NWIN - 1:
                        emit_epilogue(s)

            qn = 0
            for w in range(cfg.NWIN):
                tw, ncw = ntiles[w], ncross[w]
                sa, cross = plan.tile_sa[w], plan.tile_cross[w]
                ci_of = np.cumsum(cross) - 1
                for g0 in range(0, tw, cfg.OH_GROUP):
                    g1 = min(g0 + cfg.OH_GROUP, tw)
                    ng = g1 - g0
                    # per-group idx load: small first load -> fast start,
                    # and the pool prefetches the next group's indices
                    idx_sb = ipool.tile([P, ng * 8], i16, tag="idx")
                    nc.sync.dma_start(out=idx_sb[:],
                                      in_=idx_t[w][:, g0 * 8:g1 * 8])
                    oh = opool.tile([P, ng * P], f8, tag="oh")
                    nc.sync.dma_start(out=oh[:],
                                      in_=oha_t[w][:, g0 * P:g1 * P])
                    gcts = [t for t in range(g0, g1) if cross[t]]
                    ohb = None
                    gb0 = 0
                    if gcts:
                        gb0, gb1 = ci_of[gcts[0]], ci_of[gcts[-1]] + 1
                        ohb = obpool.tile([P, (gb1 - gb0) * P], f8, tag="ohb")
                        nc.sync.dma_start(
                            out=ohb[:],
                            in_=ohb_t[w][:, gb0 * P:gb1 * P])
                    for t0 in range(g0, g1, cfg.MAX_CHUNK_TILES):
                        t1 = min(t0 + cfg.MAX_CHUNK_TILES, g1)
                        nt = t1 - t0
                        ne = nt * P
                        msg = mpool.tile([P, nt * D_IN], bf16, tag="msg")
                        nc.gpsimd.dma_gather(
                            msg[:].rearrange("p (t f) -> p t f", f=D_IN),
                            xw[w * win:(w + 1) * win, :],
                            idx_sb[:, (t0 - g0) * 8:(t1 - g0) * 8],
                            ne, ne, D_IN,
                            queue_num=qn)
                        qn = (qn + 1) % cfg.NQUEUES
                        for t in range(t0, t1):
                            s = int(sa[t])
                            emit_mm(w, t, "A", s, msg, t0, oh, t - g0)
                            if cross[t]:
                                emit_mm(w, t, "B", s + 1, msg, t0, ohb,
                                        int(ci_of[t]) - int(gb0))

            assert not ps_open, ps_open
    nc.compile()
    return nc


# ---------------------------------------------------------------- entry

def run(cfg: Cfg, input_feat, weight, cj, ci, src_idx, dst_idx, **run_kwargs):
    plan, per_core, block_of = shard_edges(cfg, src_idx, dst_idx)
    shared, civs = host_inputs(cfg, input_feat, weight, cj, ci, block_of)
    nc = build_nc(cfg, plan)
    in_maps = []
    for c in range(NCORES):
        m = dict(shared)
        m["civ"] = civs[c]
        m.update(per_core[c])
        in_maps.append(m)
    res = run_bass_kernel_spmd(nc, in_maps, core_ids=list(range(NCORES)),
                               **run_kwargs)
    full = np.zeros((NCORES * cfg.nblk * P, cfg.D_OUT), dtype=np.float32)
    blk_rows = full.reshape(NCORES * cfg.nblk, P, cfg.D_OUT)
    for c in range(NCORES):
        o = res.results[c]["out"].reshape(P, cfg.nblk, cfg.D_OUT)
        blk_rows[block_of[c]] = o.transpose(1, 0, 2)
    return full[:cfg.N], res


def kernel(input_feat, weight, cj, ci, src_idx, dst_idx):
    out, _ = run(CFG, input_feat, weight, cj, ci, src_idx, dst_idx)
    return out


# revision 30
# speedup vs baseline: 1.1736x; 1.1736x over previous
"""GCMC GraphConv kernel for 8 Trainium2 NeuronCores.

Computation:  out = ci * segment_sum((input_feat @ weight * cj)[src], dst)

Strategy v3 (aggregate-then-transform, dst-sharded, crossing-split tiles):
  - Algebra: out = ci * (A^T (cj*X)) W  — aggregate raw (cj-scaled) X rows per
    destination and multiply by W once per dst block at the end.
  - Nodes are 1D-partitioned by destination: core c owns 98 dst blocks of
    128 rows, assigned by sorted edge count for balance.
  - The per-edge gather (SWDGE dma_gather, 256B rows, 4 queues, 1024-desc
    calls) is descriptor-execution bound at ~2.6 ns/desc aggregate — so the
    dominant cost is proportional to the PADDED edge count.  v2 padded every
    (window, slot) group to a multiple of 128 (+24%).  v3 pads each group
    only to the max count over the 8 cores (+~6%) and lets matmul tiles span
    two adjacent slots: a "crossing" tile issues TWO scatter matmuls, one per
    slot, with complementary one-hot masks (rows of the other slot get
    dstb=-1 and match nothing).  The crossing structure is canonical (derived
    from the shared K[w][s] = max_c counts), so the SPMD program is identical
    across cores; only the idx/dstb data differs.
  - Scatter is a one-hot matmul accumulated TRANSPOSED into PSUM per slot,
    flushed into accT [128 feat, 98*128 dst] f32 after each (window, slot).
  - v4: the one-hots are NOT built on-device (the DVE is_equal builds ran at
    1x rate due to the broadcast operand and were a 73%-busy co-bottleneck
    that also backpressured the gather pipeline).  Instead the host
    precomputes them as fp8_e4m3 streams (128 B/row, 27 MB/core) DMAed from
    HBM; the scatter matmul runs mixed bf16 x fp8 (legal on TRN2's PE).
  - Epilogue per dst block: matmul(lhsT=accT block, rhs=W) -> [128, 64],
    scale by ci, store.
"""

import dataclasses
import math

import numpy as np
import ml_dtypes

import concourse.bacc as bacc
import concourse.mybir as mybir
import concourse.tile as tile
from concourse.bass_utils import run_bass_kernel_spmd

BF16 = ml_dtypes.bfloat16
F8 = ml_dtypes.float8_e4m3
P = 128
NCORES = 8
D_IN = 128


@dataclasses.dataclass(frozen=True)
class Cfg:
    N: int = 100000
    D_OUT: int = 64
    NWIN: int = 4            # src windows; N/NWIN must be < 32768 (int16 idx)
    MAX_CHUNK_TILES: int = 8   # gather chunk (1024 descs = SWDGE ring capacity)
    OH_GROUP: int = 64       # tiles per one-hot stream load (8 KB/partition)
    NQUEUES: int = 4         # SWDGE queues; round-robin -> concurrent rings
    SCRATCH: int = 32768     # dynamic DMA descriptor carveout (bytes/partition)

    @property
    def n_loc(self):
        return self.N // NCORES

    @property
    def nblk(self):
        return math.ceil(self.n_loc / P)

    @property
    def win(self):
        return self.N // self.NWIN


CFG = Cfg()


@dataclasses.dataclass
class Plan:
    """Canonical (core-independent) program structure."""
    off: np.ndarray        # [NWIN, nblk+1] canonical subgroup offsets
    wtot: np.ndarray       # [NWIN] padded window sizes (x128)
    ntiles: np.ndarray     # [NWIN] tiles per window
    tile_sa: list          # per window: [T_w] primary slot of each tile
    tile_cross: list       # per window: [T_w] bool, tile spans sa and sa+1
    touches: list          # per window: {slot: [(tile, 'A'|'B')]}


def make_plan(cfg: Cfg, K: np.ndarray) -> Plan:
    nblk = cfg.nblk
    off = np.zeros((cfg.NWIN, nblk + 1), dtype=np.int64)
    off[:, 1:] = np.cumsum(K, axis=1)
    wtot = ((off[:, -1] + P - 1) // P) * P
    ntiles = wtot // P
    tile_sa, tile_cross, touches = [], [], []
    for w in range(cfg.NWIN):
        assert (K[w] >= P).all(), "subgroup smaller than a tile"
        t = int(ntiles[w])
        pos0 = np.arange(t) * P
        pos1 = np.minimum(pos0 + P - 1, off[w, -1] - 1)
        sa = np.clip(np.searchsorted(off[w], pos0, side="right") - 1,
                     0, nblk - 1)
        sb = np.clip(np.searchsorted(off[w], pos1, side="right") - 1,
                     0, nblk - 1)
        assert (sb - sa <= 1).all() and (sb >= sa).all()
        cross = sb > sa
        tile_sa.append(sa)
        tile_cross.append(cross)
        tch = {s: [] for s in range(nblk)}
        for ti in range(t):
            tch[int(sa[ti])].append((ti, "A"))
            if cross[ti]:
                tch[int(sa[ti]) + 1].append((ti, "B"))
        touches.append(tch)
    return Plan(off, wtot, ntiles, tile_sa, tile_cross, touches)


# ---------------------------------------------------------------- host prep

def shard_edges(cfg: Cfg, src, dst):
    """Route and sort edges; build canonical plan + per-core padded arrays.

    Returns (plan, per_core, block_of).
    """
    nblk, win, nw = cfg.nblk, cfg.win, cfg.NWIN
    src = np.asarray(src, dtype=np.int64)
    dst = np.asarray(dst, dtype=np.int64)
    gb = dst >> 7
    dstb = (dst & 127).astype(np.float32)
    wine = src // win
    src_loc = (src - wine * win).astype(np.int16)

    nblk_g = NCORES * nblk
    bc = np.bincount(gb, minlength=nblk_g)
    order = np.argsort(-bc, kind="stable")
    block_of = np.empty((NCORES, nblk), dtype=np.int64)
    block_core = np.empty(nblk_g, dtype=np.int64)
    block_slot = np.empty(nblk_g, dtype=np.int64)
    for s in range(nblk):
        grp = order[s * NCORES:(s + 1) * NCORES]
        block_of[:, s] = grp
        block_core[grp] = np.arange(NCORES)
        block_slot[grp] = s

    core = block_core[gb]
    slot = block_slot[gb]

    gid = (core * nw + wine) * nblk + slot
    counts = np.bincount(gid, minlength=NCORES * nw * nblk)
    counts = counts.reshape(NCORES, nw, nblk)
    K = counts.max(axis=0)                    # [NWIN, nblk] canonical sizes
    plan = make_plan(cfg, K)

    per_core = []
    for c in range(NCORES):
        m = core == c
        sl, db, we, bl = src_loc[m], dstb[m], wine[m], slot[m]
        key = we * nblk + bl
        o = np.argsort(key * (win + 1) + sl, kind="stable")
        ks = key[o]
        gcnt = np.bincount(ks, minlength=nw * nblk)
        gstart = np.concatenate([[0], np.cumsum(gcnt)[:-1]])
        within = np.arange(ks.size) - gstart[ks]
        wsel, ssel = ks // nblk, ks % nblk
        pos = plan.off[wsel, ssel] + within
        sls, dbs = sl[o], db[o].astype(np.int64)
        maps = {}
        for w in range(nw):
            n = int(plan.wtot[w])
            nt = n // P
            ia = np.zeros(n, dtype=np.int16)          # pad -> row 0
            oa = np.zeros((n, P), dtype=np.uint8)     # fp8 one-hot bits
            sel = wsel == w
            p, s_e, d_e = pos[sel], ssel[sel], dbs[sel]
            ia[p] = sls[sel]
            sa_of = plan.tile_sa[w][p // P]
            in_a = s_e == sa_of
            oa[p[in_a], d_e[in_a]] = 0x38             # fp8_e4m3 1.0
            # B-stream: rows whose slot is the tile's secondary slot
            cross = plan.tile_cross[w]
            cidx = np.cumsum(cross) - 1
            ncross = int(cross.sum())
            ob = np.zeros((max(ncross, 1) * P, P), dtype=np.uint8)
            nb = ~in_a
            tt = p[nb] // P
            assert (s_e[nb] == sa_of[nb] + 1).all()
            assert cross[tt].all()
            ob[cidx[tt] * P + (p[nb] % P), d_e[nb]] = 0x38
            maps[f"idx{w}"] = np.ascontiguousarray(
                np.tile(ia.reshape(-1, 16).T, (8, 1)))
            # [rows, cols] -> [128 row-in-tile, tiles*128 cols]
            maps[f"ohA{w}"] = np.ascontiguousarray(
                oa.reshape(nt, P, P).transpose(1, 0, 2)
                .reshape(P, nt * P)).view(F8)
            maps[f"ohB{w}"] = np.ascontiguousarray(
                ob.reshape(-1, P, P).transpose(1, 0, 2)
                .reshape(P, -1)).view(F8)
        per_core.append(maps)
    return plan, per_core, block_of


def host_inputs(cfg: Cfg, input_feat, weight, cj, ci, block_of):
    """Shared (replicated) device inputs + per-core civ (slot layout)."""
    N, nblk = cfg.N, cfg.nblk
    xs = (np.asarray(input_feat, dtype=np.float32)
          * np.asarray(cj, dtype=np.float32)).astype(BF16)
    xw = np.ascontiguousarray(xs)
    wgt = np.ascontiguousarray(np.asarray(weight, dtype=np.float32))
    cip = np.zeros(NCORES * nblk * P, dtype=np.float32)
    cif = np.asarray(ci, dtype=np.float32).reshape(-1)
    cip[:N] = cif
    cip = cip.reshape(NCORES * nblk, P)
    civs = [np.ascontiguousarray(cip[block_of[c]].T) for c in range(NCORES)]
    return {"xw": xw, "wgt": wgt}, civs


# ---------------------------------------------------------------- device IR

def build_nc(cfg: Cfg, plan: Plan):
    f32, bf16, i16 = mybir.dt.float32, mybir.dt.bfloat16, mybir.dt.int16
    f8 = mybir.dt.float8e4
    dout, nblk, win = cfg.D_OUT, cfg.nblk, cfg.win
    ntiles = [int(plan.ntiles[w]) for w in range(cfg.NWIN)]
    ncross = [int(plan.tile_cross[w].sum()) for w in range(cfg.NWIN)]

    nc = bacc.Bacc("TRN2", target_bir_lowering=False, debug=False,
                   num_swdge_queues=cfg.NQUEUES,
                   dynamic_dma_scratch_size=cfg.SCRATCH)
    xw = nc.dram_tensor("xw", [cfg.N, D_IN], bf16, kind="ExternalInput")
    wgt = nc.dram_tensor("wgt", [D_IN, dout], f32, kind="ExternalInput")
    civ = nc.dram_tensor("civ", [P, nblk], f32, kind="ExternalInput")
    idx_t = [nc.dram_tensor(f"idx{w}", [P, ntiles[w] * 8], i16,
                            kind="ExternalInput") for w in range(cfg.NWIN)]
    oha_t = [nc.dram_tensor(f"ohA{w}", [P, ntiles[w] * P], f8,
                            kind="ExternalInput") for w in range(cfg.NWIN)]
    ohb_t = [nc.dram_tensor(f"ohB{w}", [P, max(ncross[w], 1) * P], f8,
                            kind="ExternalInput") for w in range(cfg.NWIN)]
    # partition-major output: [dst-in-block, slot*dout] — contiguous per
    # partition so the final store is 1 descriptor/partition; the host
    # transposes back (it unpermutes blocks anyway).
    out_t = nc.dram_tensor("out", [P, nblk * dout], f32,
                           kind="ExternalOutput")

    # per-window slot bookkeeping: first/last touch per slot
    first_touch = []   # per window: {(tile, which): slot}  -> start flag
    last_touch = []
    for w in range(cfg.NWIN):
        ft, lt = {}, {}
        for s, tl in plan.touches[w].items():
            ft[(tl[0][0], tl[0][1])] = s
            lt[(tl[-1][0], tl[-1][1])] = s
        first_touch.append(ft)
        last_touch.append(lt)

    with tile.TileContext(nc) as tc:
        with (
            tc.tile_pool(name="const", bufs=1) as cpool,
            tc.tile_pool(name="idx", bufs=4) as ipool,
            tc.tile_pool(name="msg", bufs=12) as mpool,
            tc.tile_pool(name="oh", bufs=3) as opool,
            tc.tile_pool(name="ohb", bufs=3) as obpool,
            tc.tile_pool(name="ps", bufs=4, space="PSUM") as pspool,
            tc.tile_pool(name="pso", bufs=4, space="PSUM") as psopool,
            tc.tile_pool(name="acc", bufs=1) as apool,
        ):
            wgt_sb = cpool.tile([P, dout], f32, tag="wgt")
            nc.sync.dma_start(out=wgt_sb[:], in_=wgt[:])
            ci_sb = cpool.tile([P, nblk], f32, tag="ci")
            nc.sync.dma_start(out=ci_sb[:], in_=civ[:])
            accT = apool.tile([P, nblk * P], f32, tag="accT")
            nc.vector.memset(accT[:], 0.0)
            out_sb = apool.tile([P, nblk * dout], f32, tag="out")

            ps_open = {}

            def emit_epilogue(s):
                pso = psopool.tile([P, dout], f32, tag="pso", name="pso")
                nc.tensor.matmul(
                    out=pso[:],
                    lhsT=accT[:, s * P:(s + 1) * P],
                    rhs=wgt_sb[:],
                    start=True, stop=True)
                nc.scalar.mul(
                    out_sb[:, s * dout:(s + 1) * dout],
                    pso[:],
                    ci_sb[:, s:s + 1])
                # store completed output ranges eagerly (shrinks the tail)
                if s in (31, 63, 89, nblk - 1):
                    q0 = {31: 0, 63: 32, 89: 64, nblk - 1: 90}[s]
                    nc.sync.dma_start(
                        out=out_t[:, q0 * dout:(s + 1) * dout],
                        in_=out_sb[:, q0 * dout:(s + 1) * dout])

            def emit_mm(w, t, which, s, msg, t0, oh, ohcol):
                start = first_touch[w].get((t, which)) == s
                stop = last_touch[w].get((t, which)) == s
                if start:
                    ps_open[s] = pspool.tile([P, P], f32, tag="psT",
                                             name="psT")
                ps = ps_open[s]
                nc.tensor.matmul(
                    out=ps[:],
                    lhsT=msg[:, (t - t0) * D_IN:(t - t0 + 1) * D_IN],
                    rhs=oh[:, ohcol * P:(ohcol + 1) * P],
                    start=start, stop=stop)
                if stop:
                    nc.vector.tensor_add(
                        out=accT[:, s * P:(s + 1) * P],
                        in0=accT[:, s * P:(s + 1) * P],
                        in1=ps[:])
                    del ps_open[s]
                    if w == cfg.NWIN - 1:
                        emit_epilogue(s)

            qn = 0
            for w in range(cfg.NWIN):
                tw, ncw = ntiles[w], ncross[w]
                sa, cross = plan.tile_sa[w], plan.tile_cross[w]
                ci_of = np.cumsum(cross) - 1
                for g0 in range(0, tw, cfg.OH_GROUP):
                    g1 = min(g0 + cfg.OH_GROUP, tw)
                    ng = g1 - g0
                    # per-group idx load: small first load -> fast start,
                    # and the pool prefetches the next group's indices
                    idx_sb = ipool.tile([P, ng * 8], i16, tag="idx")
                    nc.sync.dma_start(out=idx_sb[:],
                                      in_=idx_t[w][:, g0 * 8:g1 * 8])
                    oh = opool.tile([P, ng * P], f8, tag="oh")
                    nc.sync.dma_start(out=oh[:],
                                      in_=oha_t[w][:, g0 * P:g1 * P])
                    gcts = [t for t in range(g0, g1) if cross[t]]
                    ohb = None
                    gb0 = 0
                    if gcts:
                        gb0, gb1 = ci_of[gcts[0]], ci_of[gcts[-1]] + 1
                        ohb = obpool.tile([P, (gb1 - gb0) * P], f8, tag="ohb")
                        nc.sync.dma_start(
                            out=ohb[:],
                            in_=ohb_t[w][:, gb0 * P:gb1 * P])
                    for t0 in range(g0, g1, cfg.MAX_CHUNK_TILES):
                        t1 = min(t0 + cfg.MAX_CHUNK_TILES, g1)
                        nt = t1 - t0
                        ne = nt * P
                        msg = mpool.tile([P, nt * D_IN], bf16, tag="msg")
                        nc.gpsimd.dma_gather(
                            msg[:].rearrange("p (t f) -> p t f", f=D_IN),
                            xw[w * win:(w + 1) * win, :],
                            idx_sb[:, (t0 - g0) * 8:(t1 - g0) * 8],
                            ne, ne, D_IN,
                            queue_num=qn)
                        qn = (qn + 1) % cfg.NQUEUES
                        for t in range(t0, t1):
                            s = int(sa[t])
                            emit_mm(w, t, "A", s, msg, t0, oh, t - g0)
                            if cross[t]:
                                emit_mm(w, t, "B", s + 1, msg, t0, ohb,
                                        int(ci_of[t]) - int(gb0))

            assert not ps_open, ps_open
    nc.compile()
    return nc


# ---------------------------------------------------------------- entry

def run(cfg: Cfg, input_feat, weight, cj, ci, src_idx, dst_idx, **run_kwargs):
    plan, per_core, block_of = shard_edges(cfg, src_idx, dst_idx)
    shared, civs = host_inputs(cfg, input_feat, weight, cj, ci, block_of)
    nc = build_nc(cfg, plan)
    in_maps = []
    for c in range(NCORES):
        m = dict(shared)
        m["civ"] = civs[c]
        m.update(per_core[c])
        in_maps.append(m)
    res = run_bass_kernel_spmd(nc, in_maps, core_ids=list(range(NCORES)),
                               **run_kwargs)
    full = np.zeros((NCORES * cfg.nblk * P, cfg.D_OUT), dtype=np.float32)
    blk_rows = full.reshape(NCORES * cfg.nblk, P, cfg.D_OUT)
    for c in range(NCORES):
        o = res.results[c]["out"].reshape(P, cfg.nblk, cfg.D_OUT)
        blk_rows[block_of[c]] = o.transpose(1, 0, 2)
    return full[:cfg.N], res


def kernel(input_feat, weight, cj, ci, src_idx, dst_idx):
    out, _ = run(CFG, input_feat, weight, cj, ci, src_idx, dst_idx)
    return out


# revision 31
# speedup vs baseline: 1.1992x; 1.0219x over previous
"""GCMC GraphConv kernel for 8 Trainium2 NeuronCores.

Computation:  out = ci * segment_sum((input_feat @ weight * cj)[src], dst)

Strategy v3 (aggregate-then-transform, dst-sharded, crossing-split tiles):
  - Algebra: out = ci * (A^T (cj*X)) W  — aggregate raw (cj-scaled) X rows per
    destination and multiply by W once per dst block at the end.
  - Nodes are 1D-partitioned by destination: core c owns 98 dst blocks of
    128 rows, assigned by sorted edge count for balance.
  - The per-edge gather (SWDGE dma_gather, 256B rows, 4 queues, 1024-desc
    calls) is descriptor-execution bound at ~2.6 ns/desc aggregate — so the
    dominant cost is proportional to the PADDED edge count.  v2 padded every
    (window, slot) group to a multiple of 128 (+24%).  v3 pads each group
    only to the max count over the 8 cores (+~6%) and lets matmul tiles span
    two adjacent slots: a "crossing" tile issues TWO scatter matmuls, one per
    slot, with complementary one-hot masks (rows of the other slot get
    dstb=-1 and match nothing).  The crossing structure is canonical (derived
    from the shared K[w][s] = max_c counts), so the SPMD program is identical
    across cores; only the idx/dstb data differs.
  - Scatter is a one-hot matmul accumulated TRANSPOSED into PSUM per slot,
    flushed into accT [128 feat, 98*128 dst] f32 after each (window, slot).
  - v4: the one-hots are NOT built on-device (the DVE is_equal builds ran at
    1x rate due to the broadcast operand and were a 73%-busy co-bottleneck
    that also backpressured the gather pipeline).  Instead the host
    precomputes them as fp8_e4m3 streams (128 B/row, 27 MB/core) DMAed from
    HBM; the scatter matmul runs mixed bf16 x fp8 (legal on TRN2's PE).
  - Epilogue per dst block: matmul(lhsT=accT block, rhs=W) -> [128, 64],
    scale by ci, store.
"""

import dataclasses
import math

import numpy as np
import ml_dtypes

import concourse.bacc as bacc
import concourse.mybir as mybir
import concourse.tile as tile
from concourse.bass_utils import run_bass_kernel_spmd

BF16 = ml_dtypes.bfloat16
F8 = ml_dtypes.float8_e4m3
P = 128
NCORES = 8
D_IN = 128


@dataclasses.dataclass(frozen=True)
class Cfg:
    N: int = 100000
    D_OUT: int = 64
    NWIN: int = 4            # src windows; N/NWIN must be < 32768 (int16 idx)
    MAX_CHUNK_TILES: int = 8   # gather chunk (1024 descs = SWDGE ring capacity)
    OH_GROUP: int = 64       # tiles per one-hot stream load (8 KB/partition)
    NQUEUES: int = 4         # SWDGE queues; round-robin -> concurrent rings
    SCRATCH: int = 32768     # dynamic DMA descriptor carveout (bytes/partition)

    @property
    def n_loc(self):
        return self.N // NCORES

    @property
    def nblk(self):
        return math.ceil(self.n_loc / P)

    @property
    def win(self):
        return self.N // self.NWIN


CFG = Cfg()


@dataclasses.dataclass
class Plan:
    """Canonical (core-independent) program structure."""
    off: np.ndarray        # [NWIN, nblk+1] canonical subgroup offsets
    wtot: np.ndarray       # [NWIN] padded window sizes (x128)
    ntiles: np.ndarray     # [NWIN] tiles per window
    tile_sa: list          # per window: [T_w] primary slot of each tile
    tile_cross: list       # per window: [T_w] bool, tile spans sa and sa+1
    touches: list          # per window: {slot: [(tile, 'A'|'B')]}


def make_plan(cfg: Cfg, K: np.ndarray) -> Plan:
    nblk = cfg.nblk
    off = np.zeros((cfg.NWIN, nblk + 1), dtype=np.int64)
    off[:, 1:] = np.cumsum(K, axis=1)
    wtot = ((off[:, -1] + P - 1) // P) * P
    ntiles = wtot // P
    tile_sa, tile_cross, touches = [], [], []
    for w in range(cfg.NWIN):
        assert (K[w] >= P).all(), "subgroup smaller than a tile"
        t = int(ntiles[w])
        pos0 = np.arange(t) * P
        pos1 = np.minimum(pos0 + P - 1, off[w, -1] - 1)
        sa = np.clip(np.searchsorted(off[w], pos0, side="right") - 1,
                     0, nblk - 1)
        sb = np.clip(np.searchsorted(off[w], pos1, side="right") - 1,
                     0, nblk - 1)
        assert (sb - sa <= 1).all() and (sb >= sa).all()
        cross = sb > sa
        tile_sa.append(sa)
        tile_cross.append(cross)
        tch = {s: [] for s in range(nblk)}
        for ti in range(t):
            tch[int(sa[ti])].append((ti, "A"))
            if cross[ti]:
                tch[int(sa[ti]) + 1].append((ti, "B"))
        touches.append(tch)
    return Plan(off, wtot, ntiles, tile_sa, tile_cross, touches)


# ---------------------------------------------------------------- host prep

def shard_edges(cfg: Cfg, src, dst):
    """Route and sort edges; build canonical plan + per-core padded arrays.

    Returns (plan, per_core, block_of).
    """
    nblk, win, nw = cfg.nblk, cfg.win, cfg.NWIN
    src = np.asarray(src, dtype=np.int64)
    dst = np.asarray(dst, dtype=np.int64)
    gb = dst >> 7
    dstb = (dst & 127).astype(np.float32)
    wine = src // win
    src_loc = (src - wine * win).astype(np.int16)

    nblk_g = NCORES * nblk
    bc = np.bincount(gb, minlength=nblk_g)
    order = np.argsort(-bc, kind="stable")
    block_of = np.empty((NCORES, nblk), dtype=np.int64)
    block_core = np.empty(nblk_g, dtype=np.int64)
    block_slot = np.empty(nblk_g, dtype=np.int64)
    for s in range(nblk):
        grp = order[s * NCORES:(s + 1) * NCORES]
        block_of[:, s] = grp
        block_core[grp] = np.arange(NCORES)
        block_slot[grp] = s

    core = block_core[gb]
    slot = block_slot[gb]

    gid = (core * nw + wine) * nblk + slot
    counts = np.bincount(gid, minlength=NCORES * nw * nblk)
    counts = counts.reshape(NCORES, nw, nblk)
    K = counts.max(axis=0)                    # [NWIN, nblk] canonical sizes
    plan = make_plan(cfg, K)

    per_core = []
    for c in range(NCORES):
        m = core == c
        sl, db, we, bl = src_loc[m], dstb[m], wine[m], slot[m]
        key = we * nblk + bl
        o = np.argsort(key * (win + 1) + sl, kind="stable")
        ks = key[o]
        gcnt = np.bincount(ks, minlength=nw * nblk)
        gstart = np.concatenate([[0], np.cumsum(gcnt)[:-1]])
        within = np.arange(ks.size) - gstart[ks]
        wsel, ssel = ks // nblk, ks % nblk
        pos = plan.off[wsel, ssel] + within
        sls, dbs = sl[o], db[o].astype(np.int64)
        maps = {}
        for w in range(nw):
            n = int(plan.wtot[w])
            nt = n // P
            ia = np.zeros(n, dtype=np.int16)          # pad -> row 0
            oa = np.zeros((n, P), dtype=np.uint8)     # fp8 one-hot bits
            sel = wsel == w
            p, s_e, d_e = pos[sel], ssel[sel], dbs[sel]
            ia[p] = sls[sel]
            sa_of = plan.tile_sa[w][p // P]
            in_a = s_e == sa_of
            oa[p[in_a], d_e[in_a]] = 0x38             # fp8_e4m3 1.0
            # B-stream: rows whose slot is the tile's secondary slot
            cross = plan.tile_cross[w]
            cidx = np.cumsum(cross) - 1
            ncross = int(cross.sum())
            ob = np.zeros((max(ncross, 1) * P, P), dtype=np.uint8)
            nb = ~in_a
            tt = p[nb] // P
            assert (s_e[nb] == sa_of[nb] + 1).all()
            assert cross[tt].all()
            ob[cidx[tt] * P + (p[nb] % P), d_e[nb]] = 0x38
            maps[f"idx{w}"] = np.ascontiguousarray(
                np.tile(ia.reshape(-1, 16).T, (8, 1)))
            # [rows, cols] -> [128 row-in-tile, tiles*128 cols]
            maps[f"ohA{w}"] = np.ascontiguousarray(
                oa.reshape(nt, P, P).transpose(1, 0, 2)
                .reshape(P, nt * P)).view(F8)
            maps[f"ohB{w}"] = np.ascontiguousarray(
                ob.reshape(-1, P, P).transpose(1, 0, 2)
                .reshape(P, -1)).view(F8)
        per_core.append(maps)
    return plan, per_core, block_of


def host_inputs(cfg: Cfg, input_feat, weight, cj, ci, block_of):
    """Shared (replicated) device inputs + per-core civ (slot layout)."""
    N, nblk = cfg.N, cfg.nblk
    xs = (np.asarray(input_feat, dtype=np.float32)
          * np.asarray(cj, dtype=np.float32)).astype(BF16)
    xw = np.ascontiguousarray(xs)
    wgt = np.ascontiguousarray(np.asarray(weight, dtype=np.float32))
    cip = np.zeros(NCORES * nblk * P, dtype=np.float32)
    cif = np.asarray(ci, dtype=np.float32).reshape(-1)
    cip[:N] = cif
    cip = cip.reshape(NCORES * nblk, P)
    civs = [np.ascontiguousarray(cip[block_of[c]].T) for c in range(NCORES)]
    return {"xw": xw, "wgt": wgt}, civs


# ---------------------------------------------------------------- device IR

def build_nc(cfg: Cfg, plan: Plan):
    f32, bf16, i16 = mybir.dt.float32, mybir.dt.bfloat16, mybir.dt.int16
    f8 = mybir.dt.float8e4
    dout, nblk, win = cfg.D_OUT, cfg.nblk, cfg.win
    ntiles = [int(plan.ntiles[w]) for w in range(cfg.NWIN)]
    ncross = [int(plan.tile_cross[w].sum()) for w in range(cfg.NWIN)]

    nc = bacc.Bacc("TRN2", target_bir_lowering=False, debug=False,
                   num_swdge_queues=cfg.NQUEUES,
                   dynamic_dma_scratch_size=cfg.SCRATCH)
    xw = nc.dram_tensor("xw", [cfg.N, D_IN], bf16, kind="ExternalInput")
    wgt = nc.dram_tensor("wgt", [D_IN, dout], f32, kind="ExternalInput")
    civ = nc.dram_tensor("civ", [P, nblk], f32, kind="ExternalInput")
    idx_t = [nc.dram_tensor(f"idx{w}", [P, ntiles[w] * 8], i16,
                            kind="ExternalInput") for w in range(cfg.NWIN)]
    oha_t = [nc.dram_tensor(f"ohA{w}", [P, ntiles[w] * P], f8,
                            kind="ExternalInput") for w in range(cfg.NWIN)]
    ohb_t = [nc.dram_tensor(f"ohB{w}", [P, max(ncross[w], 1) * P], f8,
                            kind="ExternalInput") for w in range(cfg.NWIN)]
    # partition-major output: [dst-in-block, slot*dout] — contiguous per
    # partition so the final store is 1 descriptor/partition; the host
    # transposes back (it unpermutes blocks anyway).
    out_t = nc.dram_tensor("out", [P, nblk * dout], f32,
                           kind="ExternalOutput")

    # per-window slot bookkeeping: first/last touch per slot
    first_touch = []   # per window: {(tile, which): slot}  -> start flag
    last_touch = []
    for w in range(cfg.NWIN):
        ft, lt = {}, {}
        for s, tl in plan.touches[w].items():
            ft[(tl[0][0], tl[0][1])] = s
            lt[(tl[-1][0], tl[-1][1])] = s
        first_touch.append(ft)
        last_touch.append(lt)

    with tile.TileContext(nc) as tc:
        with (
            tc.tile_pool(name="const", bufs=1) as cpool,
            tc.tile_pool(name="idx", bufs=6) as ipool,
            tc.tile_pool(name="msg", bufs=14) as mpool,
            tc.tile_pool(name="oh", bufs=3) as opool,
            tc.tile_pool(name="ohb", bufs=3) as obpool,
            tc.tile_pool(name="ps", bufs=4, space="PSUM") as pspool,
            tc.tile_pool(name="pso", bufs=4, space="PSUM") as psopool,
            tc.tile_pool(name="acc", bufs=1) as apool,
        ):
            wgt_sb = cpool.tile([P, dout], f32, tag="wgt")
            nc.sync.dma_start(out=wgt_sb[:], in_=wgt[:])
            ci_sb = cpool.tile([P, nblk], f32, tag="ci")
            nc.sync.dma_start(out=ci_sb[:], in_=civ[:])
            accT = apool.tile([P, nblk * P], f32, tag="accT")
            nc.vector.memset(accT[:], 0.0)
            out_sb = apool.tile([P, nblk * dout], f32, tag="out")

            ps_open = {}

            def emit_epilogue(s):
                pso = psopool.tile([P, dout], f32, tag="pso", name="pso")
                nc.tensor.matmul(
                    out=pso[:],
                    lhsT=accT[:, s * P:(s + 1) * P],
                    rhs=wgt_sb[:],
                    start=True, stop=True)
                nc.scalar.mul(
                    out_sb[:, s * dout:(s + 1) * dout],
                    pso[:],
                    ci_sb[:, s:s + 1])
                # store completed output ranges eagerly (shrinks the tail)
                if s in (31, 63, 89, nblk - 1):
                    q0 = {31: 0, 63: 32, 89: 64, nblk - 1: 90}[s]
                    nc.sync.dma_start(
                        out=out_t[:, q0 * dout:(s + 1) * dout],
                        in_=out_sb[:, q0 * dout:(s + 1) * dout])

            def emit_mm(w, t, which, s, msg, t0, oh, ohcol):
                start = first_touch[w].get((t, which)) == s
                stop = last_touch[w].get((t, which)) == s
                if start:
                    ps_open[s] = pspool.tile([P, P], f32, tag="psT",
                                             name="psT")
                ps = ps_open[s]
                nc.tensor.matmul(
                    out=ps[:],
                    lhsT=msg[:, (t - t0) * D_IN:(t - t0 + 1) * D_IN],
                    rhs=oh[:, ohcol * P:(ohcol + 1) * P],
                    start=start, stop=stop)
                if stop:
                    nc.vector.tensor_add(
                        out=accT[:, s * P:(s + 1) * P],
                        in0=accT[:, s * P:(s + 1) * P],
                        in1=ps[:])
                    del ps_open[s]
                    if w == cfg.NWIN - 1:
                        emit_epilogue(s)

            qn = 0
            for w in range(cfg.NWIN):
                tw, ncw = ntiles[w], ncross[w]
                sa, cross = plan.tile_sa[w], plan.tile_cross[w]
                ci_of = np.cumsum(cross) - 1
                for g0 in range(0, tw, cfg.OH_GROUP):
                    g1 = min(g0 + cfg.OH_GROUP, tw)
                    ng = g1 - g0
                    # per-group idx load: small first load -> fast start,
                    # and the pool prefetches the next group's indices
                    idx_sb = ipool.tile([P, ng * 8], i16, tag="idx")
                    nc.sync.dma_start(out=idx_sb[:],
                                      in_=idx_t[w][:, g0 * 8:g1 * 8])
                    oh = opool.tile([P, ng * P], f8, tag="oh")
                    nc.sync.dma_start(out=oh[:],
                                      in_=oha_t[w][:, g0 * P:g1 * P])
                    gcts = [t for t in range(g0, g1) if cross[t]]
                    ohb = None
                    gb0 = 0
                    if gcts:
                        gb0, gb1 = ci_of[gcts[0]], ci_of[gcts[-1]] + 1
                        ohb = obpool.tile([P, (gb1 - gb0) * P], f8, tag="ohb")
                        nc.sync.dma_start(
                            out=ohb[:],
                            in_=ohb_t[w][:, gb0 * P:gb1 * P])
                    for t0 in range(g0, g1, cfg.MAX_CHUNK_TILES):
                        t1 = min(t0 + cfg.MAX_CHUNK_TILES, g1)
                        nt = t1 - t0
                        ne = nt * P
                        msg = mpool.tile([P, nt * D_IN], bf16, tag="msg")
                        nc.gpsimd.dma_gather(
                            msg[:].rearrange("p (t f) -> p t f", f=D_IN),
                            xw[w * win:(w + 1) * win, :],
                            idx_sb[:, (t0 - g0) * 8:(t1 - g0) * 8],
                            ne, ne, D_IN,
                            queue_num=qn)
                        qn = (qn + 1) % cfg.NQUEUES
                        for t in range(t0, t1):
                            s = int(sa[t])
                            emit_mm(w, t, "A", s, msg, t0, oh, t - g0)
                            if cross[t]:
                                emit_mm(w, t, "B", s + 1, msg, t0, ohb,
                                        int(ci_of[t]) - int(gb0))

            assert not ps_open, ps_open
    nc.compile()
    return nc


# ---------------------------------------------------------------- entry

def run(cfg: Cfg, input_feat, weight, cj, ci, src_idx, dst_idx, **run_kwargs):
    plan, per_core, block_of = shard_edges(cfg, src_idx, dst_idx)
    shared, civs = host_inputs(cfg, input_feat, weight, cj, ci, block_of)
    nc = build_nc(cfg, plan)
    in_maps = []
    for c in range(NCORES):
        m = dict(shared)
        m["civ"] = civs[c]
        m.update(per_core[c])
        in_maps.append(m)
    res = run_bass_kernel_spmd(nc, in_maps, core_ids=list(range(NCORES)),
                               **run_kwargs)
    full = np.zeros((NCORES * cfg.nblk * P, cfg.D_OUT), dtype=np.float32)
    blk_rows = full.reshape(NCORES * cfg.nblk, P, cfg.D_OUT)
    for c in range(NCORES):
        o = res.results[c]["out"].reshape(P, cfg.nblk, cfg.D_OUT)
        blk_rows[block_of[c]] = o.transpose(1, 0, 2)
    return full[:cfg.N], res


def kernel(input_feat, weight, cj, ci, src_idx, dst_idx):
    out, _ = run(CFG, input_feat, weight, cj, ci, src_idx, dst_idx)
    return out
